# revision 1
# baseline (speedup 1.0000x reference)
"""Trainium2 Bass kernel for hyperbolic linear-attention transformer layer.

Data-parallel over nodes (N=32768) across 8 NeuronCores. Per core:
  Phase A: k/v head projections (PE, fp32r), phi_k nonlinearity (DVE/ACT),
           per-core partial ktv = phi_k^T v accumulated in PSUM, partial
           sum(phi_k) accumulated on DVE.
  AllReduce of [ktv | sumk] partials (2.1 MB) across the 8 cores.
  Phase B: q projection, phi_q, denominator folded into a per-(head,node)
           scale, attn^T computed feature-major (so the final projection
           needs no transposes), fused v_map path (W_vm = v_map_w @ Wv
           precomputed on host), final projection + Lorentz lift.

All matmuls run as float32r (full PE rate at moving-dim>=256).
"""

import os
import numpy as np
import concourse.bass as bass
import concourse.tile as tile
from concourse import bacc, mybir
from concourse.bass_utils import run_bass_kernel_spmd

F32 = mybir.dt.float32
F32R = mybir.dt.float32r
AF = mybir.ActivationFunctionType
ALU = mybir.AluOpType

NCORES = 8
N = 32768
NCHUNK = N // NCORES          # 4096 nodes per core
H = 8
D = 256
HD = H * D                    # 2048
KC = 3                        # contraction chunks: 384 = 3*128 (257 used)
EPS = 1e-6

_CACHE = {}


def _build(reps=1):
    if reps in _CACHE:
        return _CACHE[reps]
    onecore = bool(os.environ.get("KT_ONECORE"))
    nc = bacc.Bacc("TRN2", target_bir_lowering=False, debug=False,
                   num_devices=1 if onecore else NCORES)

    xqT = nc.dram_tensor("xqT", [KC, 128, NCHUNK], F32R, kind="ExternalInput").ap()
    xsT = nc.dram_tensor("xsT", [KC, 128, NCHUNK], F32R, kind="ExternalInput").ap()
    wq = nc.dram_tensor("wq", [KC, 128, HD], F32R, kind="ExternalInput").ap()
    wk = nc.dram_tensor("wk", [KC, 128, HD], F32R, kind="ExternalInput").ap()
    wv = nc.dram_tensor("wv", [KC, 128, HD], F32R, kind="ExternalInput").ap()
    wvm = nc.dram_tensor("wvm", [KC, 128, HD], F32R, kind="ExternalInput").ap()
    fw = nc.dram_tensor("fw", [16, 128, D], F32R, kind="ExternalInput").ap()
    fbias = nc.dram_tensor("fbias", [1, D], F32R, kind="ExternalInput").ap()
    ones_r = nc.dram_tensor("ones_r", [1, 128], F32R, kind="ExternalInput").ap()
    ones_c = nc.dram_tensor("ones_c", [128, 8], F32R, kind="ExternalInput").ap()
    ind = nc.dram_tensor("ind", [128, 8, 8], F32R, kind="ExternalInput").ap()
    ind2 = nc.dram_tensor("ind2", [8, 8, 128], F32R, kind="ExternalInput").ap()
    zt = nc.dram_tensor("zt", [128, 16, 8], F32R, kind="ExternalInput").ap()
    cons = nc.dram_tensor("cons", [8, 1], F32, kind="ExternalInput").ap()
    out = nc.dram_tensor("out", [NCHUNK, 257], F32, kind="ExternalOutput").ap()

    with tile.TileContext(nc) as tc:
        _body(nc, tc, reps, xqT, xsT, wq, wk, wv, wvm, fw, fbias,
              ones_r, ones_c, ind, ind2, zt, cons, out)
    nc.compile()
    _CACHE[reps] = nc
    return nc


def _body(nc, tc, reps, xqT, xsT, wq, wk, wv, wvm, fw, fbias,
          ones_r, ones_c, ind, ind2, zt, cons, out):
    import contextlib
    stack = contextlib.ExitStack()
    with stack:
        cpool = stack.enter_context(tc.tile_pool(name="const", bufs=1))
        dpool = stack.enter_context(tc.tile_pool(name="dram", bufs=1, space="DRAM"))

        ones_r_sb = cpool.tile([1, 128], F32R)
        nc.sync.dma_start(ones_r_sb[:], ones_r[:])
        ones_c_sb = cpool.tile([128, 8], F32R)
        nc.sync.dma_start(ones_c_sb[:], ones_c[:])
        ind_sb = cpool.tile([128, 8, 8], F32R)
        nc.sync.dma_start(ind_sb[:], ind[:])
        ind2_sb = cpool.tile([8, 8, 128], F32R)
        nc.sync.dma_start(ind2_sb[:], ind2[:])
        fb_sb = cpool.tile([1, D], F32R)
        nc.sync.dma_start(fb_sb[:], fbias[:])
        eps_sb = cpool.tile([8, 1], F32)
        nc.sync.dma_start(eps_sb[:], cons[:])

        ar_in = dpool.tile([129, 4096], F32)
        ar_out = dpool.tile([129, 4096], F32)

        for rep in range(reps):
            if not os.environ.get("KT_SKIP_A"):
                _phase_a(nc, tc, xsT, wk, wv, ones_c_sb, ar_in)
            if os.environ.get("KT_ONECORE"):
                nc.sync.dma_start(ar_out[:], ar_in[:])
            else:
                nc.gpsimd.collective_compute(
                    "AllReduce", ALU.add,
                    replica_groups=[list(range(NCORES))],
                    ins=[ar_in.opt()], outs=[ar_out.opt()])
            if not os.environ.get("KT_SKIP_B"):
                _phase_b(nc, tc, xqT, xsT, wq, wvm, fw, fb_sb, ones_r_sb,
                         ind_sb, ind2_sb, zt, eps_sb, ar_out, out)
            else:
                obp = tc.tile_pool(name="oBtmp", bufs=1)
                with obp as ob:
                    o_sb = ob.tile([128, 257], F32)
                    nc.sync.dma_start(o_sb[:], ar_out[0:128, 0:257])
                    for t0_ in range(NCHUNK // 128):
                        nc.sync.dma_start(out[t0_ * 128:(t0_ + 1) * 128, :], o_sb[:])


def _phase_a(nc, tc, xsT, wk, wv, ones_c_sb, ar_in):
    import contextlib
    with contextlib.ExitStack() as st:
        wpool = st.enter_context(tc.tile_pool(name="wA", bufs=1))
        xp = st.enter_context(tc.tile_pool(name="xA", bufs=3))
        zp = st.enter_context(tc.tile_pool(name="zA", bufs=2))
        yp = st.enter_context(tc.tile_pool(name="yA", bufs=2))
        scrp = st.enter_context(tc.tile_pool(name="scrA", bufs=2))
        stp = st.enter_context(tc.tile_pool(name="stA", bufs=4))
        php = st.enter_context(tc.tile_pool(name="phA", bufs=2))
        vp = st.enter_context(tc.tile_pool(name="vA", bufs=2))
        drp = st.enter_context(tc.tile_pool(name="drA", bufs=2))
        pk = st.enter_context(tc.tile_pool(name="psAk", bufs=1, space="PSUM"))
        pp = st.enter_context(tc.tile_pool(name="psAp", bufs=3, space="PSUM"))
        psk = st.enter_context(tc.tile_pool(name="psAs", bufs=1, space="PSUM"))

        wk_sb = wpool.tile([128, KC, HD], F32R)
        nc.sync.dma_start(wk_sb[:], wk.rearrange("c p n -> p c n"))
        wv_sb = wpool.tile([128, KC, HD], F32R)
        nc.sync.dma_start(wv_sb[:], wv.rearrange("c p n -> p c n"))
        sumk_acc = wpool.tile([128, HD], F32R)

        ntiles = int(os.environ.get("KT_NTILES", NCHUNK // 128))
        for g in range(2):
            gofs = g * 1024
            ktv_ps = pk.tile([128, 4, 512], F32)
            for t in range(ntiles):
                xs_sb = xp.tile([128, KC, 128], F32R, tag="xs")
                nc.sync.dma_start(
                    xs_sb[:],
                    xsT[:, :, t * 128:(t + 1) * 128].rearrange("c p n -> p c n"))

                ks_ps = []
                vs_ps = []
                for blk in range(2):
                    kp_t = pp.tile([128, 512], F32, tag="projA")
                    for c in range(KC):
                        nc.tensor.matmul(
                            kp_t[:], lhsT=xs_sb[:, c],
                            rhs=wk_sb[:, c, gofs + blk * 512: gofs + blk * 512 + 512],
                            start=(c == 0), stop=(c == KC - 1))
                    ks_ps.append(kp_t)
                for blk in range(2):
                    vp_t = pp.tile([128, 512], F32, tag="projA")
                    for c in range(KC):
                        nc.tensor.matmul(
                            vp_t[:], lhsT=xs_sb[:, c],
                            rhs=wv_sb[:, c, gofs + blk * 512: gofs + blk * 512 + 512],
                            start=(c == 0), stop=(c == KC - 1))
                    vs_ps.append(vp_t)

                # z = relu(ks) + eps
                z = zp.tile([128, 1024], F32, tag="z")
                for blk in range(2):
                    nc.vector.tensor_scalar(
                        z[:, blk * 512:(blk + 1) * 512], ks_ps[blk][:],
                        0.0, EPS, ALU.max, ALU.add)
                # v copy to SBUF (frees psum quickly)
                v_sb = vp.tile([128, 1024], F32R, tag="v")
                for blk in range(2):
                    nc.scalar.copy(v_sb[:, blk * 512:(blk + 1) * 512], vs_ps[blk][:])

                # y = z^2 with per-head accumulated sums
                y = yp.tile([128, 1024], F32R, tag="y")
                sy = stp.tile([128, 4], F32, tag="sy")
                sy2 = stp.tile([128, 4], F32, tag="sy2")
                for hh in range(4):
                    sl = slice(hh * 256, hh * 256 + 256)
                    nc.scalar.activation(y[:, sl], z[:, sl], AF.Square,
                                         accum_out=sy[:, hh:hh + 1])
                for hh in range(4):
                    sl = slice(hh * 256, hh * 256 + 256)
                    scr = scrp.tile([128, 256], F32, tag="y2scr")
                    nc.scalar.activation(scr[:], y[:, sl].bitcast(F32), AF.Square,
                                         accum_out=sy2[:, hh:hh + 1])
                # factor = sqrt(sy / sy2)
                rec = stp.tile([128, 4], F32, tag="rec")
                nc.vector.reciprocal(rec[:], sy2[:])
                rat = stp.tile([128, 4], F32, tag="rat")
                nc.vector.tensor_mul(rat[:], sy[:], rec[:])
                fac = stp.tile([128, 4], F32, tag="fac")
                nc.scalar.activation(fac[:], rat[:], AF.Sqrt)

                phi = php.tile([128, 1024], F32R, tag="phi")
                for hh in range(4):
                    sl = slice(hh * 256, hh * 256 + 256)
                    nc.vector.tensor_scalar_mul(phi[:, sl], y[:, sl].bitcast(F32),
                                                fac[:, hh:hh + 1])
                # sumk accumulation
                dst = sumk_acc[:, gofs:gofs + 1024]
                if t == 0:
                    nc.scalar.copy(dst, phi[:].bitcast(F32))
                else:
                    nc.vector.tensor_add(dst, dst.bitcast(F32), phi[:].bitcast(F32))

                # ktv accumulation: ktv[h][m,d] += phi[:,h*256+mc*128]T v[:,h*256:]
                for hh in range(4 if not os.environ.get("KT_NO_KTV") else 0):
                    for mc in range(2):
                        nc.tensor.matmul(
                            ktv_ps[:, hh, mc * 256: mc * 256 + 256],
                            lhsT=phi[:, hh * 256 + mc * 128: hh * 256 + mc * 128 + 128],
                            rhs=v_sb[:, hh * 256: hh * 256 + 256],
                            start=(t == 0), stop=(t == ntiles - 1))

            # drain ktv partials for this head group
            if not os.environ.get("KT_NO_KTV"):
                ktv_sbt = drp.tile([128, 4, 512], F32, tag="ktvdr")
                for hh in range(4):
                    nc.scalar.copy(ktv_sbt[:, hh], ktv_ps[:, hh])
                nc.sync.dma_start(ar_in[0:128, g * 2048:(g + 1) * 2048],
                                  ktv_sbt[:].rearrange("p a b -> p (a b)"))
            # sumk partition-reduction for this group
            for blk in range(2 if not os.environ.get("KT_NO_SUMK") else 0):
                sps = psk.tile([8, 512], F32, tag="sumkps")
                nc.tensor.matmul(
                    sps[:], lhsT=ones_c_sb[:],
                    rhs=sumk_acc[:, gofs + blk * 512: gofs + blk * 512 + 512],
                    start=True, stop=True)
                srow = drp.tile([1, 512], F32, tag="srow")
                nc.scalar.copy(srow[:], sps[0:1, :])
                nc.sync.dma_start(
                    ar_in[128:129, gofs + blk * 512: gofs + blk * 512 + 512],
                    srow[:])


def _phase_b(nc, tc, xqT, xsT, wq, wvm, fw, fb_sb, ones_r_sb, ind_sb, ind2_sb,
             zt, eps_sb, ar_out, out):
    import contextlib
    with contextlib.ExitStack() as st:
        wpool = st.enter_context(tc.tile_pool(name="wB", bufs=1))
        xp = st.enter_context(tc.tile_pool(name="xB", bufs=2))
        zp = st.enter_context(tc.tile_pool(name="zB", bufs=3))
        yp = st.enter_context(tc.tile_pool(name="yB", bufs=17))
        y2p = st.enter_context(tc.tile_pool(name="y2B", bufs=3))
        stp = st.enter_context(tc.tile_pool(name="stB", bufs=2))
        php = st.enter_context(tc.tile_pool(name="phB", bufs=17))
        atp = st.enter_context(tc.tile_pool(name="atB", bufs=17))
        obp = st.enter_context(tc.tile_pool(name="oB", bufs=3))
        qp = st.enter_context(tc.tile_pool(name="psBq", bufs=2, space="PSUM"))
        sump = st.enter_context(tc.tile_pool(name="psBs", bufs=1, space="PSUM"))
        sbp = st.enter_context(tc.tile_pool(name="psBb", bufs=1, space="PSUM"))
        ap_ = st.enter_context(tc.tile_pool(name="psBa", bufs=2, space="PSUM"))
        op = st.enter_context(tc.tile_pool(name="psBo", bufs=1, space="PSUM"))

        wq_sb = wpool.tile([128, KC, HD], F32R)
        nc.sync.dma_start(wq_sb[:], wq.rearrange("c p n -> p c n"))
        wvm_sb = wpool.tile([128, KC, HD], F32R)
        nc.sync.dma_start(wvm_sb[:], wvm.rearrange("c p n -> p c n"))
        fw_sb = wpool.tile([128, 16, D], F32R)
        nc.sync.dma_start(fw_sb[:], fw.rearrange("c p n -> p c n"))
        # ktv (all-reduced) as lhsT chunks [m_loc, h, mc, dc, d_loc]
        ktv_sb = wpool.tile([128, H, 2, 2, 128], F32R)
        nc.gpsimd.dma_start(
            ktv_sb[:],
            ar_out[0:128, :].rearrange("p (h mc dc dl) -> p h mc dc dl",
                                       h=H, mc=2, dc=2))
        # sumk chunk columns: [128, 16, 8], chunk c -> column h(c)
        sumk_w = wpool.tile([128, 16, 8], F32R)
        nc.sync.dma_start(sumk_w[:], zt[:])
        for c in range(16):
            hh = c // 2
            nc.gpsimd.dma_start(
                sumk_w[:, c, hh:hh + 1],
                ar_out[128:129, c * 128:(c + 1) * 128].rearrange(
                    "r (p o) -> (r p) o", o=1))

        NST = 256                      # supertile node count
        nst = int(os.environ.get("KT_NST", NCHUNK // NST))
        for stx in range(nst):
            nofs = stx * NST
            xq_sb = xp.tile([128, KC, NST], F32R, tag="xq")
            nc.sync.dma_start(
                xq_sb[:], xqT[:, :, nofs:nofs + NST].rearrange("c p n -> p c n"))
            xs_sb = xp.tile([128, KC, NST], F32R, tag="xsB")
            nc.sync.dma_start(
                xs_sb[:], xsT[:, :, nofs:nofs + NST].rearrange("c p n -> p c n"))

            sums_ps = sump.tile([8, 3, NST], F32, tag="sums")
            ys = []
            for c in range(16):
                hh = c // 2
                q_ps = qp.tile([128, NST], F32, tag="qps")
                for kc in range(KC):
                    nc.tensor.matmul(
                        q_ps[:], lhsT=wq_sb[:, kc, c * 128:(c + 1) * 128],
                        rhs=xq_sb[:, kc], start=(kc == 0), stop=(kc == KC - 1))
                z = zp.tile([128, NST], F32, tag="zB")
                nc.vector.tensor_scalar(z[:], q_ps[:], 0.0, EPS, ALU.max, ALU.add)
                y_c = yp.tile([128, NST], F32R, tag="yB")
                nc.scalar.activation(y_c[:], z[:], AF.Square)
                y2 = y2p.tile([128, NST], F32R, tag="y2B")
                nc.scalar.activation(y2[:], y_c[:].bitcast(F32), AF.Square)
                nc.tensor.matmul(sums_ps[:, 0], lhsT=ind_sb[:, hh], rhs=y_c[:],
                                 start=(c == 0), stop=(c == 15))
                nc.tensor.matmul(sums_ps[:, 1], lhsT=ind_sb[:, hh], rhs=y2[:],
                                 start=(c == 0), stop=(c == 15))
                nc.tensor.matmul(sums_ps[:, 2], lhsT=sumk_w[:, c], rhs=y_c[:],
                                 start=(c == 0), stop=(c == 15))
                ys.append(y_c)

            # stats on [8, NST]
            rec2 = stp.tile([8, NST], F32, tag="rec2")
            nc.vector.reciprocal(rec2[:], sums_ps[:, 1])
            rat = stp.tile([8, NST], F32, tag="ratB")
            nc.vector.tensor_mul(rat[:], sums_ps[:, 0], rec2[:])
            fac = stp.tile([8, NST], F32, tag="facB")
            nc.scalar.activation(fac[:], rat[:], AF.Sqrt)
            den = stp.tile([8, NST], F32, tag="den")
            nc.vector.tensor_mul(den[:], sums_ps[:, 2], fac[:])
            nc.vector.tensor_scalar_add(den[:], den[:], eps_sb[:])
            rden = stp.tile([8, NST], F32, tag="rden")
            nc.vector.reciprocal(rden[:], den[:])
            s_sb = stp.tile([8, NST], F32R, tag="sB")
            nc.vector.tensor_mul(s_sb[:], fac[:], rden[:])

            # phi' = y * s (s broadcast across partitions via K=1 matmul)
            phis = []
            for hh in range(8):
                sbc = sbp.tile([128, NST], F32, tag="sbc")
                nc.tensor.matmul(sbc[:], lhsT=ind2_sb[:, hh], rhs=s_sb[:],
                                 start=True, stop=True)
                for mc in range(2):
                    phi_c = php.tile([128, NST], F32R, tag="phB")
                    nc.vector.tensor_mul(phi_c[:], ys[2 * hh + mc][:].bitcast(F32),
                                         sbc[:])
                    phis.append(phi_c)

            # attnT chunks: attnT[(h,dc)] = sum_mc ktv[h,mc,dc]^T phi[(h,mc)] + vssT
            ats = []
            for c in range(16):
                hh, dc = c // 2, c % 2
                at_ps = ap_.tile([128, NST], F32, tag="atps")
                for mc in range(2):
                    nc.tensor.matmul(at_ps[:], lhsT=ktv_sb[:, hh, mc, dc],
                                     rhs=phis[2 * hh + mc][:],
                                     start=(mc == 0), stop=False)
                for kc in range(KC):
                    nc.tensor.matmul(at_ps[:], lhsT=wvm_sb[:, kc, c * 128:(c + 1) * 128],
                                     rhs=xs_sb[:, kc],
                                     start=False, stop=(kc == KC - 1))
                at_sb = atp.tile([128, NST], F32R, tag="atB")
                nc.scalar.copy(at_sb[:], at_ps[:])
                ats.append(at_sb)

            # final projection per 128-node subtile + Lorentz lift
            for sn in range(NST // 128):
                o_ps = op.tile([128, D], F32, tag="ops")
                for c in range(16):
                    nc.tensor.matmul(o_ps[:], lhsT=ats[c][:, sn * 128:(sn + 1) * 128],
                                     rhs=fw_sb[:, c], start=(c == 0), stop=False)
                nc.tensor.matmul(o_ps[:], lhsT=ones_r_sb[:], rhs=fb_sb[:],
                                 start=False, stop=True)
                sq = zp.tile([128, D], F32, tag="sqB")
                ssum = stp.tile([128, 1], F32, tag="ssum")
                nc.scalar.activation(sq[:], o_ps[:], AF.Square,
                                     accum_out=ssum[:])
                tcol = stp.tile([128, 1], F32, tag="tcol")
                nc.scalar.activation(tcol[:], ssum[:], AF.Sqrt, bias=1.0)
                o_sb = obp.tile([128, 257], F32, tag="osb")
                nc.vector.tensor_copy(o_sb[:, 1:257], o_ps[:])
                nc.vector.tensor_copy(o_sb[:, 0:1], tcol[:])
                nc.sync.dma_start(out[nofs + sn * 128: nofs + (sn + 1) * 128, :],
                                  o_sb[:])


def _prep_inputs(query_input, source_input, Wq_w, Wq_b, Wk_w, Wk_b, Wv_w, Wv_b,
                 norm_scale, v_map_w, v_map_b, final_w, final_b):
    def pad_x(x):
        xt = np.zeros((KC * 128, N), np.float32)
        xt[0:257] = x.T
        xt[257] = 1.0
        return xt.reshape(KC, 128, N)

    def pad_w(w_flat, b_flat):
        wt = np.zeros((KC * 128, HD), np.float32)
        wt[0:257] = w_flat.T
        wt[257] = b_flat
        return wt.reshape(KC, 128, HD)

    xq = pad_x(np.asarray(query_input))
    xs = pad_x(np.asarray(source_input))
    wq_h = pad_w(np.asarray(Wq_w).reshape(HD, 257), np.asarray(Wq_b).reshape(HD))
    wk_h = pad_w(np.asarray(Wk_w).reshape(HD, 257), np.asarray(Wk_b).reshape(HD))
    wv_h = pad_w(np.asarray(Wv_w).reshape(HD, 257), np.asarray(Wv_b).reshape(HD))

    vm = np.asarray(v_map_w)
    # wvm_flat[h] = vm @ Wv_w[h]  -> [H, 256, 257]
    wvm_flat = np.einsum('od,hdi->hoi', vm, np.asarray(Wv_w))
    bvm = (np.asarray(Wv_b) @ vm.T + np.asarray(v_map_b)[None, :]).reshape(HD)
    wvm_h = pad_w(wvm_flat.reshape(HD, 257), bvm)

    fw_h = np.ascontiguousarray(np.asarray(final_w).T).reshape(16, 128, D)
    fb_h = np.asarray(final_b).reshape(1, D).astype(np.float32)

    s = abs(float(np.asarray(norm_scale))) + EPS
    eps_eff = EPS * s * s
    cons = np.full((8, 1), eps_eff, np.float32)

    ind = np.zeros((128, 8, 8), np.float32)
    for hh in range(8):
        ind[:, hh, hh] = 1.0
    ind2 = np.zeros((8, 8, 128), np.float32)
    for hh in range(8):
        ind2[hh, hh, :] = 1.0

    common = {
        "wq": wq_h, "wk": wk_h, "wv": wv_h, "wvm": wvm_h,
        "fw": fw_h.astype(np.float32), "fbias": fb_h,
        "ones_r": np.ones((1, 128), np.float32),
        "ones_c": np.ones((128, 8), np.float32),
        "ind": ind, "ind2": ind2, "zt": np.zeros((128, 16, 8), np.float32),
        "cons": cons,
    }
    in_maps = []
    for c in range(NCORES):
        m = dict(common)
        m["xqT"] = np.ascontiguousarray(xq[:, :, c * NCHUNK:(c + 1) * NCHUNK])
        m["xsT"] = np.ascontiguousarray(xs[:, :, c * NCHUNK:(c + 1) * NCHUNK])
        in_maps.append(m)
    return in_maps


def kernel(reps=1, **inputs):
    nc = _build(reps)
    in_maps = _prep_inputs(**inputs)
    res = run_bass_kernel_spmd(nc, in_maps, list(range(NCORES)))
    return np.concatenate([res.results[c]["out"] for c in range(NCORES)], axis=0)



# revision 31
# speedup vs baseline: 1211.6513x; 1211.6513x over previous
"""Trainium2 Bass kernel for hyperbolic linear-attention transformer layer.

Data-parallel over nodes (N=32768) across 8 NeuronCores. Per core:
  Phase A (per head-group g of 4 heads): k/v projections (PE, fp32r),
    phi_k = fp(relu(k)+eps) with per-head norm stats on DVE
    (tensor_tensor_reduce) and phi scaling on ACT (Copy with scale AP),
    partial ktv = phi_k^T v accumulated in PSUM, partial sum(phi_k)
    accumulated on GpSimd. Each group's [ktv | sumk] partial (1.05 MB) is
    all-reduced separately so AR(g0) overlaps compute of g1.
  Phase B: q projection feature-major in 512-node supertiles. Because the
    denominator is ~6e4 >> eps, the fp() normalization of phi_q cancels
    between numerator and denominator, so phi_q is replaced by
    y = (relu(q)+eps)^2 with no norm chain. attn^T = ktv^T y scaled by the
    per-(node,head) reciprocal denominator; the v_map path and all biases
    are folded on the host into a single G matrix applied from x_s inside
    the final projection (homogeneous coordinate carries the bias).

All matmuls run as float32r (full PE rate at moving-dim >= 256).
"""

import os
import numpy as np
import jax
from jax.sharding import Mesh, PartitionSpec
from jax.experimental.shard_map import shard_map

import concourse.bass as bass
import concourse.tile as tile
import concourse.mybir as mybir
from concourse import bacc, bass2jax

F32 = mybir.dt.float32
F32R = mybir.dt.float32r
AF = mybir.ActivationFunctionType
ALU = mybir.AluOpType

NCORES = 8
N = 32768
NCHUNK = N // NCORES          # 4096 nodes per core
H = 8
D = 256
HD = H * D                    # 2048
KC = 3                        # contraction chunks: 384 = 3*128 (258 used)
NST = 512                     # phase-B supertile node count
EPS = 1e-6

_CACHE = {}
_EXEC_CACHE = {}


def _build(reps=1):
    if reps in _CACHE:
        return _CACHE[reps]
    onecore = bool(os.environ.get("KT_ONECORE"))
    nc = bacc.Bacc("TRN2", target_bir_lowering=False, debug=False,
                   num_devices=1 if onecore else NCORES)

    xqT = nc.dram_tensor("xqT", [KC, 128, NCHUNK], F32R, kind="ExternalInput").ap()
    xsT = nc.dram_tensor("xsT", [KC, 128, NCHUNK], F32R, kind="ExternalInput").ap()
    wq = nc.dram_tensor("wq", [KC, 128, HD], F32R, kind="ExternalInput").ap()
    wk = nc.dram_tensor("wk", [KC, 128, HD], F32R, kind="ExternalInput").ap()
    wv = nc.dram_tensor("wv", [KC, 128, HD], F32R, kind="ExternalInput").ap()
    gw = nc.dram_tensor("gw", [KC, 128, D], F32R, kind="ExternalInput").ap()
    fw = nc.dram_tensor("fw", [16, 128, D], F32R, kind="ExternalInput").ap()
    ones_c = nc.dram_tensor("ones_c", [128, 8], F32R, kind="ExternalInput").ap()
    ind2 = nc.dram_tensor("ind2", [8, 8, 128], F32R, kind="ExternalInput").ap()
    zt = nc.dram_tensor("zt", [128, 16, 8], F32R, kind="ExternalInput").ap()
    cons = nc.dram_tensor("cons", [8, 1], F32, kind="ExternalInput").ap()
    out = nc.dram_tensor("out", [NCHUNK, 257], F32, kind="ExternalOutput").ap()

    with tile.TileContext(nc) as tc:
        _body(nc, tc, reps, xqT, xsT, wq, wk, wv, gw, fw,
              ones_c, ind2, zt, cons, out)
    nc.compile()
    _CACHE[reps] = nc
    return nc


def _body(nc, tc, reps, xqT, xsT, wq, wk, wv, gw, fw, ones_c, ind2, zt, cons, out):
    import contextlib
    stack = contextlib.ExitStack()
    with stack:
        cpool = stack.enter_context(tc.tile_pool(name="const", bufs=1))
        dpool = stack.enter_context(tc.tile_pool(name="dram", bufs=1, space="DRAM"))
        # persistent B-phase weights: allocated outside the A pools so their
        # DMAs don't WAR-serialize behind phase A's SBUF reuse
        bwp = stack.enter_context(tc.tile_pool(name="wB", bufs=1))

        ones_c_sb = cpool.tile([128, 8], F32R)
        nc.sync.dma_start(ones_c_sb[:], ones_c[:])
        zrow_sb = cpool.tile([1, 2 * HD], F32)
        nc.vector.memset(zrow_sb[:], 0.0)
        ind2_sb = cpool.tile([8, 8, 128], F32R)
        nc.sync.dma_start(ind2_sb[:], ind2[:])
        cons_sb = cpool.tile([8, 1], F32)
        nc.sync.dma_start(cons_sb[:], cons[:])

        out_space = "Shared" if os.environ.get("KT_SHARED") else "Local"
        if not os.environ.get("KT_DUALAR"):
            arm_in = dpool.tile([129, 2 * HD], F32, tag="armi", name="arm_in")
            arm_out = dpool.tile([129, 2 * HD], F32, tag="armo", name="arm_out")
            ar_in = [arm_in, arm_in]
            ar_out = [arm_out, arm_out]
            ar_co = [0, HD]
        else:
            ar_in = [dpool.tile([129, HD], F32, tag=f"ari{g}", name=f"ar_in{g}")
                     for g in range(2)]
            ar_out = [dpool.tile([129, HD], F32, tag=f"aro{g}", name=f"ar_out{g}",
                                 addr_space=out_space)
                      for g in range(2)]
            ar_co = [0, 0]

        # the sumk row is only partially written per group; zero it once so
        # the collective never reduces uninitialized memory
        if ar_in[0] is ar_in[1]:
            nc.sync.dma_start(ar_in[0][128:129, :], zrow_sb[:, 0:2 * HD])
        else:
            for g in range(2):
                nc.sync.dma_start(ar_in[g][128:129, :], zrow_sb[:, 0:HD])

        for rep in range(reps):
            if not os.environ.get("KT_SKIP_A"):
                with nc.named_scope(f"phase_a_{rep}"):
                    _phase_a(nc, tc, xsT, wk, wv, ones_c_sb, ar_in, ar_out, ar_co)
            if not os.environ.get("KT_SKIP_B"):
                with nc.named_scope(f"phase_b_{rep}"):
                    _phase_b(nc, tc, xqT, xsT, wq, gw, fw, ind2_sb, zt, cons_sb,
                             ar_out, ar_co, out, bwp)
            else:
                with tc.tile_pool(name="oBtmp", bufs=1) as ob:
                    o_sb = ob.tile([128, 257], F32)
                    nc.vector.memset(o_sb[:], 0.0)
                    for t0_ in range(NCHUNK // 128):
                        nc.sync.dma_start(out[t0_ * 128:(t0_ + 1) * 128, :], o_sb[:])


def _phase_a(nc, tc, xsT, wk, wv, ones_c_sb, ar_in, ar_out, ar_co):
    import contextlib
    with contextlib.ExitStack() as st:
        wpool = st.enter_context(tc.tile_pool(name="wA", bufs=1))
        xp = st.enter_context(tc.tile_pool(name="xA", bufs=3))
        zp = st.enter_context(tc.tile_pool(name="zA", bufs=2))
        yp = st.enter_context(tc.tile_pool(name="yA", bufs=2))
        scrp = st.enter_context(tc.tile_pool(name="scrA", bufs=2))
        stp = st.enter_context(tc.tile_pool(name="stA", bufs=4))
        php = st.enter_context(tc.tile_pool(name="phA", bufs=3))
        vp = st.enter_context(tc.tile_pool(name="vA", bufs=3))
        drp = st.enter_context(tc.tile_pool(name="drA", bufs=2))
        pk = st.enter_context(tc.tile_pool(name="psAk", bufs=1, space="PSUM"))
        pp = st.enter_context(tc.tile_pool(name="psAp", bufs=1, space="PSUM"))

        wk_sb = wpool.tile([128, KC, HD], F32R)
        nc.sync.dma_start(wk_sb[:], wk.rearrange("c p n -> p c n"))
        wv_sb = wpool.tile([128, KC, HD], F32R)
        nc.sync.dma_start(wv_sb[:], wv.rearrange("c p n -> p c n"))
        sumk_acc = wpool.tile([128, HD], F32R)

        ntiles = NCHUNK // 128
        for g in range(2):
            gofs = g * 1024
            ktv_ps = pk.tile([128, 4, 512], F32, tag="ktv")
            for t in range(ntiles):
                xs_sb = xp.tile([128, KC, 128], F32R, tag="xs")
                nc.sync.dma_start(
                    xs_sb[:],
                    xsT[:, :, t * 128:(t + 1) * 128].rearrange("c p n -> p c n"))

                kp_t = pp.tile([128, 1024], F32, tag="kp")
                for blk in range(2):
                    for c in range(KC):
                        nc.tensor.matmul(
                            kp_t[:, blk * 512:(blk + 1) * 512], lhsT=xs_sb[:, c],
                            rhs=wk_sb[:, c, gofs + blk * 512: gofs + blk * 512 + 512],
                            start=(c == 0), stop=(c == KC - 1))
                vp_t = pp.tile([128, 1024], F32, tag="vp")
                for blk in range(2):
                    for c in range(KC):
                        nc.tensor.matmul(
                            vp_t[:, blk * 512:(blk + 1) * 512], lhsT=xs_sb[:, c],
                            rhs=wv_sb[:, c, gofs + blk * 512: gofs + blk * 512 + 512],
                            start=(c == 0), stop=(c == KC - 1))

                # z = relu(k) + eps   (DVE, reads PSUM)
                z = zp.tile([128, 1024], F32, tag="z")
                for blk in range(2):
                    nc.vector.tensor_scalar(
                        z[:, blk * 512:(blk + 1) * 512], kp_t[:, blk * 512:(blk + 1) * 512],
                        0.0, EPS, ALU.max, ALU.add)
                # v copy to SBUF (frees psum; split DVE/ACT)
                v_sb = vp.tile([128, 1024], F32R, tag="v")
                nc.vector.tensor_copy(v_sb[:, 0:512], vp_t[:, 0:512])
                nc.scalar.copy(v_sb[:, 512:1024], vp_t[:, 512:1024])

                # y = z^2 with per-head sum (ACT accum); sy2 = sum(y^2) (DVE ttr)
                y = yp.tile([128, 1024], F32R, tag="y")
                sy = stp.tile([128, 4], F32, tag="sy")
                sy2 = stp.tile([128, 4], F32, tag="sy2")
                for hh in range(4):
                    sl = slice(hh * 256, hh * 256 + 256)
                    nc.scalar.activation(y[:, sl], z[:, sl], AF.Square,
                                         accum_out=sy[:, hh:hh + 1])
                for hh in range(4):
                    sl = slice(hh * 256, hh * 256 + 256)
                    scr = scrp.tile([128, 256], F32, tag="y2scr")
                    nc.scalar.activation(scr[:], y[:, sl].bitcast(F32), AF.Square,
                                         accum_out=sy2[:, hh:hh + 1])
                # factor = sqrt(sy / sy2)
                rec = stp.tile([128, 4], F32, tag="rec")
                nc.vector.reciprocal(rec[:], sy2[:])
                rat = stp.tile([128, 4], F32, tag="rat")
                nc.vector.tensor_mul(rat[:], sy[:], rec[:])
                fac = stp.tile([128, 4], F32, tag="fac")
                nc.scalar.activation(fac[:], rat[:], AF.Sqrt)

                # phi = y * fac  (DVE per-partition scalar mul)
                phi = php.tile([128, 1024], F32R, tag="phi")
                for hh in range(4):
                    sl = slice(hh * 256, hh * 256 + 256)
                    nc.vector.tensor_scalar_mul(phi[:, sl], y[:, sl].bitcast(F32),
                                                fac[:, hh:hh + 1])
                # sumk accumulation (DVE)
                dst = sumk_acc[:, gofs:gofs + 1024]
                if t == 0:
                    nc.vector.tensor_copy(dst, phi[:].bitcast(F32))
                else:
                    nc.vector.tensor_add(dst, dst.bitcast(F32), phi[:].bitcast(F32))

                # ktv accumulation: ktv[h][m,d] += phi[:,h*256+mc*128]T v[:,h*256:]
                for hh in range(4):
                    for mc in range(2):
                        nc.tensor.matmul(
                            ktv_ps[:, hh, mc * 256: mc * 256 + 256],
                            lhsT=phi[:, hh * 256 + mc * 128: hh * 256 + mc * 128 + 128],
                            rhs=v_sb[:, hh * 256: hh * 256 + 256],
                            start=(t == 0), stop=(t == ntiles - 1),
                            skip_group_check=True)

            # drain ktv partials for this head group
            ktv_sbt = drp.tile([128, 4, 512], F32, tag="ktvdr")
            nc.vector.tensor_copy(ktv_sbt[:, 0], ktv_ps[:, 0])
            nc.vector.tensor_copy(ktv_sbt[:, 1], ktv_ps[:, 1])
            nc.scalar.copy(ktv_sbt[:, 2], ktv_ps[:, 2])
            nc.scalar.copy(ktv_sbt[:, 3], ktv_ps[:, 3])
            co = ar_co[g]
            nc.sync.dma_start(ar_in[g][0:128, co:co + HD],
                              ktv_sbt[:].rearrange("p a b -> p (a b)"))
            # sumk partition-reduction for this group
            sps = pp.tile([8, 1024], F32, tag="kp")
            for blk in range(2):
                nc.tensor.matmul(
                    sps[:, blk * 512:(blk + 1) * 512], lhsT=ones_c_sb[:],
                    rhs=sumk_acc[:, gofs + blk * 512: gofs + blk * 512 + 512],
                    start=True, stop=True)
            srow = drp.tile([1, 1024], F32, tag="srow")
            nc.vector.tensor_copy(srow[:], sps[0:1, :])
            nc.sync.dma_start(ar_in[g][128:129, co:co + 1024], srow[:])

            single_ar = not os.environ.get("KT_DUALAR")
            if os.environ.get("KT_ONECORE"):
                if not single_ar or g == 1:
                    nc.sync.dma_start(ar_out[g][:], ar_in[g][:])
            elif not single_ar or g == 1:
                nc.gpsimd.collective_compute(
                    "AllReduce", ALU.add,
                    replica_groups=[list(range(NCORES))],
                    ins=[ar_in[g].opt()], outs=[ar_out[g].opt()])


def _phase_b(nc, tc, xqT, xsT, wq, gw, fw, ind2_sb, zt, cons_sb, ar_out, ar_co,
             out, bwp):
    import contextlib
    with contextlib.ExitStack() as st:
        xp = st.enter_context(tc.tile_pool(name="xB", bufs=2))
        zp = st.enter_context(tc.tile_pool(name="zB", bufs=3))
        yp = st.enter_context(tc.tile_pool(name="yB", bufs=20))
        stp = st.enter_context(tc.tile_pool(name="stB", bufs=2))
        scp = st.enter_context(tc.tile_pool(name="scB", bufs=2))
        atp = st.enter_context(tc.tile_pool(name="atB", bufs=18))
        obp = st.enter_context(tc.tile_pool(name="oB", bufs=3))
        qp = st.enter_context(tc.tile_pool(name="psBq", bufs=2, space="PSUM"))
        sump = st.enter_context(tc.tile_pool(name="psBs", bufs=1, space="PSUM"))
        sbp = st.enter_context(tc.tile_pool(name="psBb", bufs=1, space="PSUM"))
        ap_ = st.enter_context(tc.tile_pool(name="psBa", bufs=2, space="PSUM"))
        op = st.enter_context(tc.tile_pool(name="psBo", bufs=2, space="PSUM"))

        wq_sb = bwp.tile([128, KC, HD], F32R, tag="wq")
        nc.sync.dma_start(wq_sb[:], wq.rearrange("c p n -> p c n"))
        fw_sb = bwp.tile([128, 16, D], F32R, tag="fw")
        nc.sync.dma_start(fw_sb[:], fw.rearrange("c p n -> p c n"))
        gw_sb = bwp.tile([128, KC, D], F32R, tag="gw")
        nc.sync.dma_start(gw_sb[:], gw.rearrange("c p n -> p c n"))
        # ktv (all-reduced) as lhsT chunks [m_loc, h, mc, dc, d_loc]
        ktv_sb = bwp.tile([128, H, 2, 2, 128], F32R, tag="ktv")
        for g in range(2):
            co = ar_co[g]
            nc.gpsimd.dma_start(
                ktv_sb[:, 4 * g:4 * g + 4],
                ar_out[g][0:128, co:co + HD].rearrange(
                    "p (h mc dc dl) -> p h mc dc dl", h=4, mc=2, dc=2))
        # sumk chunk columns: [128, 16, 8], chunk c -> column h(c)
        sumk_w = bwp.tile([128, 16, 8], F32R, tag="sumk")
        nc.sync.dma_start(sumk_w[:], zt[:])
        for c in range(16):
            hh = c // 2
            g, cl = c // 8, c % 8
            co = ar_co[g]
            nc.gpsimd.dma_start(
                sumk_w[:, c, hh:hh + 1],
                ar_out[g][128:129, co + cl * 128:co + (cl + 1) * 128].rearrange(
                    "r (p o) -> (r p) o", o=1))

        nst = NCHUNK // NST
        for stx in range(nst):
            nofs = stx * NST
            xq_sb = xp.tile([128, KC, NST], F32R, tag="xq")
            nc.sync.dma_start(
                xq_sb[:], xqT[:, :, nofs:nofs + NST].rearrange("c p n -> p c n"))
            xs_sb = xp.tile([128, KC, NST], F32R, tag="xsB")
            nc.sync.dma_start(
                xs_sb[:], xsT[:, :, nofs:nofs + NST].rearrange("c p n -> p c n"))

            den_ps = sump.tile([8, NST], F32, tag="den")
            ys = []
            for c in range(16):
                q_ps = qp.tile([128, NST], F32, tag="qps")
                for kc in range(KC):
                    nc.tensor.matmul(
                        q_ps[:], lhsT=wq_sb[:, kc, c * 128:(c + 1) * 128],
                        rhs=xq_sb[:, kc], start=(kc == 0), stop=(kc == KC - 1))
                z = zp.tile([128, NST], F32, tag="zB")
                nc.vector.tensor_scalar(z[:], q_ps[:], 0.0, EPS, ALU.max, ALU.add)
                y_c = yp.tile([128, NST], F32R, tag="yB")
                nc.scalar.activation(y_c[:], z[:], AF.Square)
                nc.tensor.matmul(den_ps[:], lhsT=sumk_w[:, c], rhs=y_c[:],
                                 start=(c == 0), stop=(c == 15))
                ys.append(y_c)

            den_sb = stp.tile([8, NST], F32, tag="denS")
            nc.vector.tensor_scalar_add(den_sb[:], den_ps[:], cons_sb[:])
            rden = stp.tile([8, NST], F32R, tag="rden")
            with nc.allow_low_precision(reason="f32r rounding of 1/den is fine"):
                nc.vector.reciprocal(rden[:], den_sb[:])

            # attnT chunks: attnT[(h,dc)] = [sum_mc ktv[h,mc,dc]^T y[(h,mc)]] * rden_bc
            ats = []
            for hh in range(8):
                sbc_ps = sbp.tile([128, NST], F32, tag="sbc")
                nc.tensor.matmul(sbc_ps[:], lhsT=ind2_sb[:, hh], rhs=rden[:],
                                 start=True, stop=True)
                sbc_sb = scp.tile([128, NST], F32, tag="sbcs")
                nc.scalar.copy(sbc_sb[:], sbc_ps[:])
                for dc in range(2):
                    at_ps = ap_.tile([128, NST], F32, tag="atps")
                    nc.tensor.matmul(at_ps[:], lhsT=ktv_sb[:, hh, 0, dc],
                                     rhs=ys[2 * hh][:], start=True, stop=False)
                    nc.tensor.matmul(at_ps[:], lhsT=ktv_sb[:, hh, 1, dc],
                                     rhs=ys[2 * hh + 1][:], start=False, stop=True)
                    at_sb = atp.tile([128, NST], F32R, tag="atsb")
                    nc.vector.tensor_mul(at_sb[:], at_ps[:], sbc_sb[:])
                    ats.append(at_sb)

            # final projection per 128-node subtile (+ G term, bias via homog row)
            for sn in range(NST // 128):
                o_ps = op.tile([128, D], F32, tag="ops")
                for c in range(16):
                    nc.tensor.matmul(o_ps[:], lhsT=ats[c][:, sn * 128:(sn + 1) * 128],
                                     rhs=fw_sb[:, c], start=(c == 0), stop=False)
                for kc in range(KC):
                    nc.tensor.matmul(o_ps[:], lhsT=xs_sb[:, kc, sn * 128:(sn + 1) * 128],
                                     rhs=gw_sb[:, kc],
                                     start=False, stop=(kc == KC - 1))
                sq = zp.tile([128, D], F32, tag="sqB")
                ssum = stp.tile([128, 1], F32, tag="ssum")
                nc.scalar.activation(sq[:], o_ps[:], AF.Square,
                                     accum_out=ssum[:])
                tcol = stp.tile([128, 1], F32, tag="tcol")
                nc.scalar.activation(tcol[:], ssum[:], AF.Sqrt, bias=1.0)
                o_sb = obp.tile([128, 257], F32, tag="osb")
                nc.vector.tensor_copy(o_sb[:, 1:257], o_ps[:])
                nc.vector.tensor_copy(o_sb[:, 0:1], tcol[:])
                nc.sync.dma_start(out[nofs + sn * 128: nofs + (sn + 1) * 128, :],
                                  o_sb[:])


def _prep_inputs(query_input, source_input, Wq_w, Wq_b, Wk_w, Wk_b, Wv_w, Wv_b,
                 norm_scale, v_map_w, v_map_b, final_w, final_b):
    def pad_x(x):
        xt = np.zeros((KC * 128, N), np.float32)
        xt[0:257] = x.T
        xt[257] = 1.0
        return xt.reshape(KC, 128, N)

    def pad_w(w_flat, b_flat, width):
        wt = np.zeros((KC * 128, width), np.float32)
        wt[0:257] = w_flat.T
        wt[257] = b_flat
        return wt.reshape(KC, 128, width)

    xq = pad_x(np.asarray(query_input, np.float32))
    xs = pad_x(np.asarray(source_input, np.float32))
    wq_h = pad_w(np.asarray(Wq_w).reshape(HD, 257), np.asarray(Wq_b).reshape(HD), HD)
    wk_h = pad_w(np.asarray(Wk_w).reshape(HD, 257), np.asarray(Wk_b).reshape(HD), HD)
    wv_h = pad_w(np.asarray(Wv_w).reshape(HD, 257), np.asarray(Wv_b).reshape(HD), HD)

    # G-fold: final_w . blockdiag(v_map) . Wv collapsed to one [D, 257] matrix
    fwm = np.asarray(final_w, np.float64)
    vm = np.asarray(v_map_w, np.float64)
    Wv = np.asarray(Wv_w, np.float64)
    bv = np.asarray(Wv_b, np.float64)
    vmb = np.asarray(v_map_b, np.float64)
    G_w = np.zeros((D, 257), np.float64)
    G_b = np.asarray(final_b, np.float64).copy()
    for h in range(H):
        F_h = fwm[:, h * D:(h + 1) * D]
        G_w += F_h @ (vm @ Wv[h])
        G_b += F_h @ (vm @ bv[h] + vmb)
    gw_h = pad_w(G_w.astype(np.float32), G_b.astype(np.float32), D)

    fw_h = np.ascontiguousarray(np.asarray(final_w, np.float32).T).reshape(16, 128, D)

    s = abs(float(np.asarray(norm_scale))) + EPS
    cons = np.full((8, 1), EPS * s * s, np.float32)

    ind2 = np.zeros((8, 8, 128), np.float32)
    for hh in range(8):
        ind2[hh, hh, :] = 1.0

    common = {
        "wq": wq_h, "wk": wk_h, "wv": wv_h, "gw": gw_h, "fw": fw_h,
        "ones_c": np.ones((128, 8), np.float32),
        "ind2": ind2, "zt": np.zeros((128, 16, 8), np.float32),
        "cons": cons,
    }
    in_maps = []
    for c in range(NCORES):
        m = dict(common)
        m["xqT"] = np.ascontiguousarray(xq[:, :, c * NCHUNK:(c + 1) * NCHUNK])
        m["xsT"] = np.ascontiguousarray(xs[:, :, c * NCHUNK:(c + 1) * NCHUNK])
        in_maps.append(m)
    return in_maps


def _build_runner(nc):
    """Build a reusable jitted PJRT executable for the bass module
    (same lowering as bass_utils.run_bass_kernel_spmd under axon, but the
    compiled executable is cached so repeat calls skip re-tracing)."""
    bass2jax.install_neuronx_cc_hook()
    partition_name = nc.partition_id_tensor.name if nc.partition_id_tensor else None
    in_names, out_names, out_avals, zero_shapes = [], [], [], []
    for alloc in nc.m.functions[0].allocations:
        if not isinstance(alloc, mybir.MemoryLocationSet):
            continue
        name = alloc.memorylocations[0].name
        if alloc.kind == "ExternalInput":
            if name != partition_name:
                in_names.append(name)
        elif alloc.kind == "ExternalOutput":
            out_names.append(name)
            shape = tuple(alloc.tensor_shape)
            dtype = mybir.dt.np(alloc.dtype)
            out_avals.append(jax.core.ShapedArray(shape, dtype))
            zero_shapes.append((shape, dtype))
    n_params = len(in_names)
    n_outs = len(out_avals)
    in_names_full = list(in_names) + out_names
    if partition_name is not None:
        in_names_full.append(partition_name)

    def _bass_body(*args):
        operands = list(args)
        if partition_name is not None:
            operands.append(bass2jax.partition_id_tensor())
        outs = bass2jax._bass_exec_p.bind(
            *operands,
            out_avals=tuple(out_avals),
            in_names=tuple(in_names_full),
            out_names=tuple(out_names),
            lowering_input_output_aliases=(),
            sim_require_finite=True,
            sim_require_nnan=True,
            nc=nc,
        )
        return tuple(outs)

    devices = jax.devices()[:NCORES]
    mesh = Mesh(np.asarray(devices), ("core",))
    in_specs = (PartitionSpec("core"),) * (n_params + n_outs)
    out_specs = (PartitionSpec("core"),) * n_outs
    donate = tuple(range(n_params, n_params + n_outs))
    fn = jax.jit(
        shard_map(_bass_body, mesh=mesh, in_specs=in_specs, out_specs=out_specs,
                  check_rep=False),
        donate_argnums=donate, keep_unused=True)
    return fn, in_names, out_names, out_avals, zero_shapes


def run_spmd_cached(nc, in_maps):
    key = id(nc)
    if key not in _EXEC_CACHE:
        _EXEC_CACHE[key] = _build_runner(nc)
    fn, in_names, out_names, out_avals, zero_shapes = _EXEC_CACHE[key]
    per_core = [[np.asarray(m[name]) for name in in_names] for m in in_maps]
    concat_in = [np.concatenate([per_core[c][i] for c in range(NCORES)], axis=0)
                 for i in range(len(in_names))]
    concat_zeros = [np.zeros((NCORES * s[0], *s[1:]), d) for (s, d) in zero_shapes]
    out_arrs = fn(*concat_in, *concat_zeros)
    return [
        {name: np.asarray(out_arrs[i]).reshape(NCORES, *out_avals[i].shape)[c]
         for i, name in enumerate(out_names)}
        for c in range(NCORES)
    ]


def kernel(reps=1, **inputs):
    nc = _build(reps)
    in_maps = _prep_inputs(**inputs)
    res = run_spmd_cached(nc, in_maps)
    return np.concatenate([res[c]["out"] for c in range(NCORES)], axis=0)
